# revision 4
# baseline (speedup 1.0000x reference)
"""MoE fusion kernel for Trainium2, data-parallel across 8 NeuronCores.

Reference computation (per row b of B=16384):
    x      = concat(z_s, z_e)                    # [1024]
    wgt    = softmax(x @ rw + rb)                # [8]
    h_e    = gelu(x @ w1[e] + b1[e])             # [8, 1024]
    y_e    = h_e @ w2[e] + b2[e]                 # [8, 1024]
    ln_e   = (y_e - mu_e) * rsqrt(var_e + eps) * gamma[e] + beta[e]
    z      = sum_e wgt[e] * ln_e                 # [1024]

Sharding: batch split 8 ways (2048 rows/core), all params replicated.
No collectives. Matmuls run in float32r (TF32-like, 1 cyc/row) with fp32
PSUM accumulation.

Per-core dataflow: activations are kept feature-major ("xT" layout
[feat, batch]) for layer 1 so the stored weight layout [in, out] can be
used directly as the stationary operand; layer 2 uses the hidden
activations as stationary, producing y in batch-major layout so the
LayerNorm reduction runs along the free dimension (bn_stats/bn_aggr).
Biases b2/rb are added inside PSUM via K=1 matmuls against a ones
vector; b1 rides the Gelu activation's per-partition bias.
"""
import numpy as np
from contextlib import ExitStack

import concourse.bass as bass
import concourse.bacc as bacc
import concourse.mybir as mybir
import concourse.tile as tile
from concourse.bass_utils import run_bass_kernel_spmd

P = 128          # partitions
D = 1024         # IN_DIM == OUT_DIM
E = 8            # experts
NK = D // P      # 8 contraction chunks
NCORES = 8
B_FULL = 16384
BL = B_FULL // NCORES   # 2048 rows per core
SEQ = 512               # z_s/z_e width

F32 = mybir.dt.float32
F32R = mybir.dt.float32r
AF = mybir.ActivationFunctionType
ALU = mybir.AluOpType


def _build(bl, st, fast_affine):
    """Build the per-core Bass program.

    bl: rows per core; st: supertile rows (multiple of 512, divides bl)
    fast_affine: True when gamma==1 and beta==0 (skips the per-expert
    affine ops; z written directly by expert 0).
    """
    nst = bl // st          # supertiles
    nt = st // 512          # 512-wide moving tiles per supertile
    nb = st // P            # 128-row chunks per supertile

    nc = bacc.Bacc(None, target_bir_lowering=False)
    zs_d = nc.declare_dram_parameter("zs", [bl, SEQ], F32, isOutput=False)
    ze_d = nc.declare_dram_parameter("ze", [bl, SEQ], F32, isOutput=False)
    rw_d = nc.declare_dram_parameter("rw", [D, E], F32R, isOutput=False)
    rb_d = nc.declare_dram_parameter("rb", [E], F32R, isOutput=False)
    w1_d = nc.declare_dram_parameter("w1", [E, D, D], F32R, isOutput=False)
    b1_d = nc.declare_dram_parameter("b1", [E, D], F32, isOutput=False)
    w2_d = nc.declare_dram_parameter("w2", [E, D, D], F32R, isOutput=False)
    b2_d = nc.declare_dram_parameter("b2", [E, D], F32R, isOutput=False)
    gam_d = nc.declare_dram_parameter("gam", [E, D], F32, isOutput=False)
    bet_d = nc.declare_dram_parameter("bet", [E, D], F32, isOutput=False)
    id_d = nc.declare_dram_parameter("ident", [P, P], F32, isOutput=False)
    on_d = nc.declare_dram_parameter("ones", [P], F32R, isOutput=False)
    z_d = nc.declare_dram_parameter("z", [bl, D], F32, isOutput=True)

    r = F32R

    with tile.TileContext(nc) as tc, ExitStack() as ctx:
        consts = ctx.enter_context(tc.tile_pool(name="consts", bufs=1))
        xload = ctx.enter_context(tc.tile_pool(name="xload", bufs=2))
        xtp = ctx.enter_context(tc.tile_pool(name="xtp", bufs=1))
        wbufs = 8 if not fast_affine else 9
        wp1 = ctx.enter_context(tc.tile_pool(name="wp1", bufs=wbufs))
        wp2 = ctx.enter_context(tc.tile_pool(name="wp2", bufs=wbufs))
        hp = ctx.enter_context(tc.tile_pool(name="hp", bufs=16 if fast_affine else 10))
        zp = ctx.enter_context(tc.tile_pool(name="zp", bufs=nb))
        cp = ctx.enter_context(tc.tile_pool(name="cp", bufs=4))
        bp = ctx.enter_context(tc.tile_pool(name="bp", bufs=2))
        wsp = ctx.enter_context(tc.tile_pool(name="wsp", bufs=nb + 2))
        sp = ctx.enter_context(tc.tile_pool(name="sp", bufs=8))
        gp = None
        if not fast_affine:
            gp = ctx.enter_context(tc.tile_pool(name="gp", bufs=2))
        psA = ctx.enter_context(tc.tile_pool(name="psA", bufs=3, space="PSUM"))
        psB = ctx.enter_context(tc.tile_pool(name="psB", bufs=4, space="PSUM"))

        ident = consts.tile([P, P], F32)
        nc.sync.dma_start(out=ident, in_=id_d[:])
        eps_t = consts.tile([P, 1], F32)
        nc.vector.memset(eps_t, 1e-5)
        ones_t = consts.tile([1, P], F32R)
        nc.sync.dma_start(out=ones_t, in_=on_d[:].rearrange("(one p) -> one p", one=1))
        rw_sb = consts.tile([P, NK, E], F32R)
        nc.sync.dma_start(out=rw_sb, in_=rw_d[:].rearrange("(c p) e -> p c e", p=P))
        rb_sb = consts.tile([1, E], F32R)
        nc.sync.dma_start(out=rb_sb, in_=rb_d[:].rearrange("(one e) -> one e", one=1))

        for s_i in range(nst):
            base = s_i * st
            # ---- transpose x supertile into feature-major xt chunks ----
            xt = [xtp.tile([P, st], F32R, tag=f"xt{c}", name=f"xt_{s_i}_{c}")
                  for c in range(NK)]
            for b in range(nb):
                x_sb = xload.tile([P, D], F32, tag="x", name=f"x_{s_i}_{b}")
                row = base + b * P
                nc.sync.dma_start(out=x_sb[:, :SEQ], in_=zs_d[row:row + P, :])
                nc.sync.dma_start(out=x_sb[:, SEQ:], in_=ze_d[row:row + P, :])
                for c in range(NK):
                    tp = psA.tile([P, P], F32, tag="a", name=f"tp_{s_i}_{b}_{c}")
                    nc.tensor.transpose(tp, x_sb[:, c * P:(c + 1) * P], ident)
                    nc.scalar.activation(out=xt[c][:, b * P:(b + 1) * P], in_=tp,
                                         func=AF.Copy)

            # ---- router: logits -> softmax weights, batch-major ----
            wsm = []
            for b in range(nb):
                ps_r = psA.tile([P, E], F32, tag="a", name=f"psr_{s_i}_{b}")
                for c in range(NK):
                    nc.tensor.matmul(ps_r, xt[c][:, b * P:(b + 1) * P],
                                     rw_sb[:, c, :],
                                     start=(c == 0), stop=False)
                nc.tensor.matmul(ps_r, ones_t, rb_sb,
                                 start=False, stop=True)
                ex = sp.tile([P, E], F32, tag="ex", name=f"ex_{s_i}_{b}")
                nc.scalar.activation(out=ex, in_=ps_r, func=AF.Exp)
                sm = sp.tile([P, 1], F32, tag="sm", name=f"sm_{s_i}_{b}")
                nc.vector.tensor_reduce(out=sm, in_=ex, axis=mybir.AxisListType.X,
                                        op=ALU.add)
                rc = sp.tile([P, 1], F32, tag="rc", name=f"rc_{s_i}_{b}")
                nc.vector.reciprocal(out=rc, in_=sm)
                wt = wsp.tile([P, E], F32, tag="wt", name=f"wt_{s_i}_{b}")
                nc.vector.tensor_scalar_mul(out=wt, in0=ex, scalar1=rc)
                wsm.append(wt)

            z_t = [zp.tile([P, D], F32, tag="z", name=f"z_{s_i}_{b}")
                   for b in range(nb)]
            if not fast_affine:
                for b in range(nb):
                    nc.vector.memset(z_t[b], 0.0)

            # ---- expert loop ----
            for e in range(E):
                w1t = [wp1.tile([P, D], F32R, tag="w1", name=f"w1_{s_i}_{e}_{c}")
                       for c in range(NK)]
                w2t = [wp2.tile([P, D], F32R, tag="w2", name=f"w2_{s_i}_{e}_{c}")
                       for c in range(NK)]
                for c in range(NK):
                    nc.sync.dma_start(out=w1t[c], in_=w1_d[e, c * P:(c + 1) * P, :])
                for c in range(NK):
                    nc.sync.dma_start(out=w2t[c], in_=w2_d[e, c * P:(c + 1) * P, :])
                b1_sb = bp.tile([P, NK], F32, tag="b1", name=f"b1_{s_i}_{e}")
                nc.sync.dma_start(out=b1_sb,
                                  in_=b1_d[e].rearrange("(m p) -> p m", p=P))
                b2_sb = bp.tile([1, D], F32R, tag="b2", name=f"b2_{s_i}_{e}")
                nc.sync.dma_start(out=b2_sb,
                                  in_=b2_d[e].rearrange("(one d) -> one d", one=1))
                if not fast_affine:
                    gam_sb = gp.tile([P, D], F32, tag="g", name=f"g_{s_i}_{e}")
                    nc.sync.dma_start(out=gam_sb,
                                      in_=gam_d[e].partition_broadcast(P))
                    bet_sb = gp.tile([P, D], F32, tag="bt", name=f"bt_{s_i}_{e}")
                    nc.sync.dma_start(out=bet_sb,
                                      in_=bet_d[e].partition_broadcast(P))

                for t in range(nt):
                    # layer 1: hT chunks [feat 128, batch 512]
                    ht = []
                    for m in range(NK):
                        ps_h = psA.tile([P, 512], F32, tag="a",
                                        name=f"ph_{s_i}_{e}_{t}_{m}")
                        for c in range(NK):
                            nc.tensor.matmul(
                                ps_h,
                                w1t[c][:, m * P:(m + 1) * P],
                                xt[c][:, t * 512:(t + 1) * 512],
                                start=(c == 0), stop=(c == NK - 1))
                        hc = hp.tile([P, 512], F32R, tag="h",
                                     name=f"h_{s_i}_{e}_{t}_{m}")
                        nc.scalar.activation(out=hc, in_=ps_h, func=AF.Gelu,
                                             bias=b1_sb[:, m:m + 1], scale=1.0)
                        ht.append(hc)

                    # layer 2 + LN + weighted accumulate, per 128-row chunk
                    for s in range(4):
                        bb = t * 4 + s
                        ys = []
                        for n in range(2):
                            ps_y = psB.tile([P, 512], F32, tag="y",
                                            name=f"py_{s_i}_{e}_{bb}_{n}")
                            for c in range(NK):
                                nc.tensor.matmul(
                                    ps_y,
                                    ht[c][:, s * P:(s + 1) * P],
                                    w2t[c][:, n * 512:(n + 1) * 512],
                                    start=(c == 0), stop=False)
                            nc.tensor.matmul(ps_y, ones_t,
                                             b2_sb[:, n * 512:(n + 1) * 512],
                                             start=False, stop=True)
                            ys.append(ps_y)
                        stats = sp.tile([P, 2, 6], F32, tag="st",
                                        name=f"st_{s_i}_{e}_{bb}")
                        nc.vector.bn_stats(out=stats[:, 0, :], in_=ys[0])
                        nc.vector.bn_stats(out=stats[:, 1, :], in_=ys[1])
                        mv = sp.tile([P, 2], F32, tag="mv", name=f"mv_{s_i}_{e}_{bb}")
                        nc.vector.bn_aggr(out=mv, in_=stats)
                        sd = sp.tile([P, 1], F32, tag="sd", name=f"sd_{s_i}_{e}_{bb}")
                        nc.scalar.activation(out=sd, in_=mv[:, 1:2], func=AF.Sqrt,
                                             bias=eps_t, scale=1.0)
                        rstd = sp.tile([P, 1], F32, tag="rs", name=f"rs_{s_i}_{e}_{bb}")
                        nc.vector.reciprocal(out=rstd, in_=sd)
                        alpha = sp.tile([P, 1], F32, tag="al", name=f"al_{s_i}_{e}_{bb}")
                        nc.vector.tensor_mul(alpha, rstd, wsm[bb][:, e:e + 1])
                        nbias = sp.tile([P, 1], F32, tag="nb", name=f"nb_{s_i}_{e}_{bb}")
                        nc.vector.tensor_scalar(out=nbias, in0=mv[:, 0:1],
                                                scalar1=alpha, scalar2=-1.0,
                                                op0=ALU.mult, op1=ALU.mult)
                        for n in range(2):
                            zslice = z_t[bb][:, n * 512:(n + 1) * 512]
                            if fast_affine and e == 0:
                                nc.scalar.activation(out=zslice, in_=ys[n],
                                                     func=AF.Identity,
                                                     bias=nbias, scale=alpha)
                            else:
                                ct = cp.tile([P, 512], F32, tag="ct",
                                             name=f"ct_{s_i}_{e}_{bb}_{n}")
                                nc.scalar.activation(out=ct, in_=ys[n],
                                                     func=AF.Identity,
                                                     bias=nbias, scale=alpha)
                                if not fast_affine:
                                    gs = gam_sb[:, n * 512:(n + 1) * 512]
                                    nc.vector.tensor_mul(ct, ct, gs)
                                    bw = cp.tile([P, 512], F32, tag="bw",
                                                 name=f"bw_{s_i}_{e}_{bb}_{n}")
                                    nc.vector.tensor_scalar_mul(
                                        out=bw,
                                        in0=bet_sb[:, n * 512:(n + 1) * 512],
                                        scalar1=wsm[bb][:, e:e + 1])
                                    nc.vector.tensor_add(ct, ct, bw)
                                nc.vector.tensor_add(zslice, zslice, ct)

            for b in range(nb):
                nc.sync.dma_start(out=z_d[base + b * P:base + (b + 1) * P, :],
                                  in_=z_t[b])

    nc.compile()
    return nc


_NC_CACHE = {}


def _get_nc(bl, st, fast_affine):
    key = (bl, st, fast_affine)
    if key not in _NC_CACHE:
        _NC_CACHE[key] = _build(bl, st, fast_affine)
    return _NC_CACHE[key]


def kernel(z_s, z_e, router_w, router_b, w1, b1, w2, b2, gamma, beta):
    z_s = np.ascontiguousarray(np.asarray(z_s, dtype=np.float32))
    z_e = np.ascontiguousarray(np.asarray(z_e, dtype=np.float32))
    router_w = np.ascontiguousarray(np.asarray(router_w, dtype=np.float32))
    router_b = np.ascontiguousarray(np.asarray(router_b, dtype=np.float32))
    w1 = np.ascontiguousarray(np.asarray(w1, dtype=np.float32))
    b1 = np.ascontiguousarray(np.asarray(b1, dtype=np.float32))
    w2 = np.ascontiguousarray(np.asarray(w2, dtype=np.float32))
    b2 = np.ascontiguousarray(np.asarray(b2, dtype=np.float32))
    gamma = np.ascontiguousarray(np.asarray(gamma, dtype=np.float32))
    beta = np.ascontiguousarray(np.asarray(beta, dtype=np.float32))

    fast_affine = bool(np.all(gamma == 1.0) and np.all(beta == 0.0))
    nc = _get_nc(BL, 1024, fast_affine)

    ident = np.eye(P, dtype=np.float32)
    ones_h = np.ones(P, dtype=np.float32)
    in_maps = []
    for c in range(NCORES):
        sl = slice(c * BL, (c + 1) * BL)
        in_maps.append({
            "zs": z_s[sl], "ze": z_e[sl],
            "rw": router_w, "rb": router_b,
            "w1": w1, "b1": b1, "w2": w2, "b2": b2,
            "gam": gamma, "bet": beta,
            "ident": ident, "ones": ones_h,
        })
    res = run_bass_kernel_spmd(nc, in_maps, core_ids=list(range(NCORES)))
    return np.concatenate([res.results[c]["z"] for c in range(NCORES)], axis=0)


# revision 5
# speedup vs baseline: 1.0543x; 1.0543x over previous
"""MoE fusion kernel for Trainium2, data-parallel across 8 NeuronCores.

Reference computation (per row b of B=16384):
    x      = concat(z_s, z_e)                    # [1024]
    wgt    = softmax(x @ rw + rb)                # [8]
    h_e    = gelu(x @ w1[e] + b1[e])             # [8, 1024]
    y_e    = h_e @ w2[e] + b2[e]                 # [8, 1024]
    ln_e   = (y_e - mu_e) * rsqrt(var_e + eps) * gamma[e] + beta[e]
    z      = sum_e wgt[e] * ln_e                 # [1024]

Sharding: batch split 8 ways (2048 rows/core), all params replicated.
No collectives. Matmuls run in float32r (TF32-like, 1 cyc/row) with fp32
PSUM accumulation.

Per-core dataflow: activations are kept feature-major ("xT" layout
[feat, batch]) for layer 1 so the stored weight layout [in, out] can be
used directly as the stationary operand; layer 2 uses the hidden
activations as stationary, producing y in batch-major layout so the
LayerNorm reduction runs along the free dimension (bn_stats/bn_aggr).
Biases b2/rb are added inside PSUM via K=1 matmuls against a ones
vector; b1 rides the Gelu activation's per-partition bias.
"""
import numpy as np
from contextlib import ExitStack

import concourse.bass as bass
import concourse.bacc as bacc
import concourse.mybir as mybir
import concourse.tile as tile
from concourse.bass_utils import run_bass_kernel_spmd

P = 128          # partitions
D = 1024         # IN_DIM == OUT_DIM
E = 8            # experts
NK = D // P      # 8 contraction chunks
NCORES = 8
B_FULL = 16384
BL = B_FULL // NCORES   # 2048 rows per core
SEQ = 512               # z_s/z_e width

F32 = mybir.dt.float32
F32R = mybir.dt.float32r
AF = mybir.ActivationFunctionType
ALU = mybir.AluOpType


def _build(bl, st, fast_affine):
    """Build the per-core Bass program.

    bl: rows per core; st: supertile rows (multiple of 512, divides bl)
    fast_affine: True when gamma==1 and beta==0 (skips the per-expert
    affine ops; z written directly by expert 0).
    """
    nst = bl // st          # supertiles
    nt = st // 512          # 512-wide moving tiles per supertile
    nb = st // P            # 128-row chunks per supertile

    nc = bacc.Bacc(None, target_bir_lowering=False)
    zs_d = nc.declare_dram_parameter("zs", [bl, SEQ], F32, isOutput=False)
    ze_d = nc.declare_dram_parameter("ze", [bl, SEQ], F32, isOutput=False)
    rw_d = nc.declare_dram_parameter("rw", [D, E], F32R, isOutput=False)
    rb_d = nc.declare_dram_parameter("rb", [E], F32R, isOutput=False)
    w1_d = nc.declare_dram_parameter("w1", [E, D, D], F32R, isOutput=False)
    b1_d = nc.declare_dram_parameter("b1", [E, D], F32, isOutput=False)
    w2_d = nc.declare_dram_parameter("w2", [E, D, D], F32R, isOutput=False)
    b2_d = nc.declare_dram_parameter("b2", [E, D], F32, isOutput=False)
    gam_d = nc.declare_dram_parameter("gam", [E, D], F32, isOutput=False)
    bet_d = nc.declare_dram_parameter("bet", [E, D], F32, isOutput=False)
    id_d = nc.declare_dram_parameter("ident", [P, P], F32, isOutput=False)
    on_d = nc.declare_dram_parameter("ones", [P], F32R, isOutput=False)
    z_d = nc.declare_dram_parameter("z", [bl, D], F32, isOutput=True)

    r = F32R

    with tile.TileContext(nc) as tc, ExitStack() as ctx:
        consts = ctx.enter_context(tc.tile_pool(name="consts", bufs=1))
        xload = ctx.enter_context(tc.tile_pool(name="xload", bufs=2))
        xtp = ctx.enter_context(tc.tile_pool(name="xtp", bufs=1))
        wbufs = 8 if not fast_affine else 9
        wp1 = ctx.enter_context(tc.tile_pool(name="wp1", bufs=wbufs))
        wp2 = ctx.enter_context(tc.tile_pool(name="wp2", bufs=wbufs))
        hp = ctx.enter_context(tc.tile_pool(name="hp", bufs=16 if fast_affine else 10))
        zp = ctx.enter_context(tc.tile_pool(name="zp", bufs=nb))
        cp = ctx.enter_context(tc.tile_pool(name="cp", bufs=4))
        bp = ctx.enter_context(tc.tile_pool(name="bp", bufs=2))
        wsp = ctx.enter_context(tc.tile_pool(name="wsp", bufs=nb + 2))
        sp = ctx.enter_context(tc.tile_pool(name="sp", bufs=8))
        gp = None
        if not fast_affine:
            gp = ctx.enter_context(tc.tile_pool(name="gp", bufs=2))
        psA = ctx.enter_context(tc.tile_pool(name="psA", bufs=4, space="PSUM"))
        psB = ctx.enter_context(tc.tile_pool(name="psB", bufs=4, space="PSUM"))

        ident = consts.tile([P, P], F32)
        nc.sync.dma_start(out=ident, in_=id_d[:])
        eps_t = consts.tile([P, 1], F32)
        nc.vector.memset(eps_t, 1e-5)
        ones_t = consts.tile([1, P], F32R)
        nc.sync.dma_start(out=ones_t, in_=on_d[:].rearrange("(one p) -> one p", one=1))
        rw_sb = consts.tile([P, NK, E], F32R)
        nc.sync.dma_start(out=rw_sb, in_=rw_d[:].rearrange("(c p) e -> p c e", p=P))
        rb_sb = consts.tile([1, E], F32R)
        nc.sync.dma_start(out=rb_sb, in_=rb_d[:].rearrange("(one e) -> one e", one=1))

        for s_i in range(nst):
            base = s_i * st
            # ---- transpose x supertile into feature-major xt chunks ----
            xt = [xtp.tile([P, st], F32R, tag=f"xt{c}", name=f"xt_{s_i}_{c}")
                  for c in range(NK)]
            for b in range(nb):
                x_sb = xload.tile([P, D], F32, tag="x", name=f"x_{s_i}_{b}")
                row = base + b * P
                nc.sync.dma_start(out=x_sb[:, :SEQ], in_=zs_d[row:row + P, :])
                nc.sync.dma_start(out=x_sb[:, SEQ:], in_=ze_d[row:row + P, :])
                for c in range(NK):
                    tp = psA.tile([P, P], F32, tag="a", name=f"tp_{s_i}_{b}_{c}")
                    nc.tensor.transpose(tp, x_sb[:, c * P:(c + 1) * P], ident)
                    nc.scalar.activation(out=xt[c][:, b * P:(b + 1) * P], in_=tp,
                                         func=AF.Copy)

            # ---- router: logits -> softmax weights, batch-major ----
            wsm = []
            for b in range(nb):
                ps_r = psA.tile([P, E], F32, tag="a", name=f"psr_{s_i}_{b}")
                for c in range(NK):
                    nc.tensor.matmul(ps_r, xt[c][:, b * P:(b + 1) * P],
                                     rw_sb[:, c, :],
                                     start=(c == 0), stop=False)
                nc.tensor.matmul(ps_r, ones_t, rb_sb,
                                 start=False, stop=True)
                ex = sp.tile([P, E], F32, tag="ex", name=f"ex_{s_i}_{b}")
                nc.scalar.activation(out=ex, in_=ps_r, func=AF.Exp)
                sm = sp.tile([P, 1], F32, tag="sm", name=f"sm_{s_i}_{b}")
                nc.vector.tensor_reduce(out=sm, in_=ex, axis=mybir.AxisListType.X,
                                        op=ALU.add)
                rc = sp.tile([P, 1], F32, tag="rc", name=f"rc_{s_i}_{b}")
                nc.vector.reciprocal(out=rc, in_=sm)
                wt = wsp.tile([P, E], F32, tag="wt", name=f"wt_{s_i}_{b}")
                nc.vector.tensor_scalar_mul(out=wt, in0=ex, scalar1=rc)
                wsm.append(wt)

            z_t = [zp.tile([P, D], F32, tag="z", name=f"z_{s_i}_{b}")
                   for b in range(nb)]
            if not fast_affine:
                for b in range(nb):
                    nc.vector.memset(z_t[b], 0.0)

            # ---- expert loop ----
            for e in range(E):
                w1t = [wp1.tile([P, D], F32R, tag="w1", name=f"w1_{s_i}_{e}_{c}")
                       for c in range(NK)]
                w2t = [wp2.tile([P, D], F32R, tag="w2", name=f"w2_{s_i}_{e}_{c}")
                       for c in range(NK)]
                for c in range(NK):
                    nc.sync.dma_start(out=w1t[c], in_=w1_d[e, c * P:(c + 1) * P, :])
                for c in range(NK):
                    nc.sync.dma_start(out=w2t[c], in_=w2_d[e, c * P:(c + 1) * P, :])
                b1_sb = bp.tile([P, NK], F32, tag="b1", name=f"b1_{s_i}_{e}")
                nc.sync.dma_start(out=b1_sb,
                                  in_=b1_d[e].rearrange("(m p) -> p m", p=P))
                b2_sb = bp.tile([P, D], F32, tag="b2", name=f"b2_{s_i}_{e}")
                nc.sync.dma_start(out=b2_sb, in_=b2_d[e].partition_broadcast(P))
                if not fast_affine:
                    gam_sb = gp.tile([P, D], F32, tag="g", name=f"g_{s_i}_{e}")
                    nc.sync.dma_start(out=gam_sb,
                                      in_=gam_d[e].partition_broadcast(P))
                    bet_sb = gp.tile([P, D], F32, tag="bt", name=f"bt_{s_i}_{e}")
                    nc.sync.dma_start(out=bet_sb,
                                      in_=bet_d[e].partition_broadcast(P))

                for t in range(nt):
                    # layer 1: hT chunks [feat 128, batch 512]
                    ht = []
                    for m in range(NK):
                        ps_h = psA.tile([P, 512], F32, tag="a",
                                        name=f"ph_{s_i}_{e}_{t}_{m}")
                        for c in range(NK):
                            nc.tensor.matmul(
                                ps_h,
                                w1t[c][:, m * P:(m + 1) * P],
                                xt[c][:, t * 512:(t + 1) * 512],
                                start=(c == 0), stop=(c == NK - 1))
                        hc = hp.tile([P, 512], F32R, tag="h",
                                     name=f"h_{s_i}_{e}_{t}_{m}")
                        nc.scalar.activation(out=hc, in_=ps_h, func=AF.Gelu,
                                             bias=b1_sb[:, m:m + 1], scale=1.0)
                        ht.append(hc)

                    # layer 2 + LN + weighted accumulate, per 128-row chunk
                    for s in range(4):
                        bb = t * 4 + s
                        ys = []
                        for n in range(2):
                            ps_y = psB.tile([P, 512], F32, tag="y",
                                            name=f"py_{s_i}_{e}_{bb}_{n}")
                            for c in range(NK):
                                nc.tensor.matmul(
                                    ps_y,
                                    ht[c][:, s * P:(s + 1) * P],
                                    w2t[c][:, n * 512:(n + 1) * 512],
                                    start=(c == 0), stop=(c == NK - 1))
                            yb = cp.tile([P, 512], F32, tag="yb",
                                         name=f"yb_{s_i}_{e}_{bb}_{n}")
                            nc.vector.tensor_add(yb, ps_y,
                                                 b2_sb[:, n * 512:(n + 1) * 512])
                            ys.append(yb)
                        stats = sp.tile([P, 2, 6], F32, tag="st",
                                        name=f"st_{s_i}_{e}_{bb}")
                        nc.vector.bn_stats(out=stats[:, 0, :], in_=ys[0])
                        nc.vector.bn_stats(out=stats[:, 1, :], in_=ys[1])
                        mv = sp.tile([P, 2], F32, tag="mv", name=f"mv_{s_i}_{e}_{bb}")
                        nc.vector.bn_aggr(out=mv, in_=stats)
                        sd = sp.tile([P, 1], F32, tag="sd", name=f"sd_{s_i}_{e}_{bb}")
                        nc.scalar.activation(out=sd, in_=mv[:, 1:2], func=AF.Sqrt,
                                             bias=eps_t, scale=1.0)
                        rstd = sp.tile([P, 1], F32, tag="rs", name=f"rs_{s_i}_{e}_{bb}")
                        nc.vector.reciprocal(out=rstd, in_=sd)
                        alpha = sp.tile([P, 1], F32, tag="al", name=f"al_{s_i}_{e}_{bb}")
                        nc.vector.tensor_mul(alpha, rstd, wsm[bb][:, e:e + 1])
                        nbias = sp.tile([P, 1], F32, tag="nb", name=f"nb_{s_i}_{e}_{bb}")
                        nc.vector.tensor_scalar(out=nbias, in0=mv[:, 0:1],
                                                scalar1=alpha, scalar2=-1.0,
                                                op0=ALU.mult, op1=ALU.mult)
                        for n in range(2):
                            zslice = z_t[bb][:, n * 512:(n + 1) * 512]
                            if fast_affine and e == 0:
                                nc.scalar.activation(out=zslice, in_=ys[n],
                                                     func=AF.Identity,
                                                     bias=nbias, scale=alpha)
                            else:
                                ct = cp.tile([P, 512], F32, tag="ct",
                                             name=f"ct_{s_i}_{e}_{bb}_{n}")
                                nc.scalar.activation(out=ct, in_=ys[n],
                                                     func=AF.Identity,
                                                     bias=nbias, scale=alpha)
                                if not fast_affine:
                                    gs = gam_sb[:, n * 512:(n + 1) * 512]
                                    nc.vector.tensor_mul(ct, ct, gs)
                                    bw = cp.tile([P, 512], F32, tag="bw",
                                                 name=f"bw_{s_i}_{e}_{bb}_{n}")
                                    nc.vector.tensor_scalar_mul(
                                        out=bw,
                                        in0=bet_sb[:, n * 512:(n + 1) * 512],
                                        scalar1=wsm[bb][:, e:e + 1])
                                    nc.vector.tensor_add(ct, ct, bw)
                                nc.vector.tensor_add(zslice, zslice, ct)

            for b in range(nb):
                nc.sync.dma_start(out=z_d[base + b * P:base + (b + 1) * P, :],
                                  in_=z_t[b])

    nc.compile()
    return nc


_NC_CACHE = {}


def _get_nc(bl, st, fast_affine):
    key = (bl, st, fast_affine)
    if key not in _NC_CACHE:
        _NC_CACHE[key] = _build(bl, st, fast_affine)
    return _NC_CACHE[key]


def kernel(z_s, z_e, router_w, router_b, w1, b1, w2, b2, gamma, beta):
    z_s = np.ascontiguousarray(np.asarray(z_s, dtype=np.float32))
    z_e = np.ascontiguousarray(np.asarray(z_e, dtype=np.float32))
    router_w = np.ascontiguousarray(np.asarray(router_w, dtype=np.float32))
    router_b = np.ascontiguousarray(np.asarray(router_b, dtype=np.float32))
    w1 = np.ascontiguousarray(np.asarray(w1, dtype=np.float32))
    b1 = np.ascontiguousarray(np.asarray(b1, dtype=np.float32))
    w2 = np.ascontiguousarray(np.asarray(w2, dtype=np.float32))
    b2 = np.ascontiguousarray(np.asarray(b2, dtype=np.float32))
    gamma = np.ascontiguousarray(np.asarray(gamma, dtype=np.float32))
    beta = np.ascontiguousarray(np.asarray(beta, dtype=np.float32))

    fast_affine = bool(np.all(gamma == 1.0) and np.all(beta == 0.0))
    nc = _get_nc(BL, 1024, fast_affine)

    ident = np.eye(P, dtype=np.float32)
    ones_h = np.ones(P, dtype=np.float32)
    in_maps = []
    for c in range(NCORES):
        sl = slice(c * BL, (c + 1) * BL)
        in_maps.append({
            "zs": z_s[sl], "ze": z_e[sl],
            "rw": router_w, "rb": router_b,
            "w1": w1, "b1": b1, "w2": w2, "b2": b2,
            "gam": gamma, "bet": beta,
            "ident": ident, "ones": ones_h,
        })
    res = run_bass_kernel_spmd(nc, in_maps, core_ids=list(range(NCORES)))
    return np.concatenate([res.results[c]["z"] for c in range(NCORES)], axis=0)
